# revision 1
# baseline (speedup 1.0000x reference)
"""Two-layer GAT (single-head GATConv x2 + log_softmax) on 8 Trainium2 cores.

Edge-parallel, dst-sharded, scatter-free. Nodes split into 8 contiguous dst
ranges; each core owns the edges targeting its range and computes its output
rows fully locally. Host preprocessing (graph structure only) sorts edges by
dst and packs them into fixed 64-dst-node windows, each padded to TPW tiles
of 128 edges, so the whole device schedule is static and identical on all
cores (SPMD); all per-core variation lives in data tensors.

Per layer: fp16 node-table rows [h | a_src-score | pad] are gathered by src
(one indirect DMA per 128-edge tile); per-dst a_dst scores stream in via a
static strided load + one selection matmul (window->tile map is static);
logits get exp(leaky_relu(.) - B) on the scalar engine (B = core-local max
bound; the shift cancels in the per-dst softmax, which is core-local, so no
collective is needed and fp16 exp retains ~e^6 headroom); messages and segment
sums accumulate across each window's TPW tiles directly in PSUM (matmul
start/stop chains against 0/1 indicators), and each completed 128-dst-row
block is copied once into an SBUF-resident fp32 accumulator — no DRAM
accumulators, no zeroing, no indirect scatters. The flush phase reads the
SBUF accumulator, divides by the segment sum, applies bias+ReLU (layer 1 ->
next table + AllGather) or bias+log_softmax (layer 2 -> output).
"""

import math
import numpy as np

import concourse.bass as bass
import concourse.mybir as mybir
from concourse.bass import IndirectOffsetOnAxis
from concourse.tile import TileContext
from concourse.masks import make_identity

FP32 = mybir.dt.float32
FP16 = mybir.dt.float16
I32 = mybir.dt.int32

CORES = 8
W = 64          # dst nodes per window (psum bases 0/64)
WPB = 2         # windows per block (128 dst rows)
BPC = 2         # blocks per chunk
WPC = WPB * BPC             # windows per chunk = 4
DST_PC = W * WPC            # dst rows per chunk = 256
GRP = 128 // W  # tiles per indt partition group = 2
WIN = 512       # dst rows per flush window (h2ext matmul width)
B_MARGIN = 5.0  # exp shift: bias = MARGIN - B so w <= e^MARGIN


# ---------------------------------------------------------------------------
# Host-side graph preprocessing (structure only)
# ---------------------------------------------------------------------------

def preprocess(edges_index: np.ndarray, n_nodes: int, n_cores: int = CORES):
    npc = int(math.ceil(n_nodes / n_cores))
    pad = int(math.ceil(npc / WIN)) * WIN
    nch = pad // DST_PC
    assert pad % WIN == 0 and pad % DST_PC == 0

    src = np.concatenate([edges_index[0], np.arange(n_nodes, dtype=np.int64)])
    dst = np.concatenate([edges_index[1], np.arange(n_nodes, dtype=np.int64)])
    order = np.argsort(dst, kind="stable")
    src = src[order]
    dst = dst[order]

    # Balance windows: nodes are freely permutable within a core (x is
    # permuted on input, output inverse-permuted on the host), so LPT-pack
    # nodes into 64-node windows by in-degree to minimize the max window
    # edge count (which sets tpw = gathers per window for ALL windows).
    import heapq
    nwin = pad // W
    slot_map = np.zeros((n_cores, pad), np.int64)
    for c in range(n_cores):
        lo = np.searchsorted(dst, c * npc, "left")
        hi = np.searchsorted(dst, min((c + 1) * npc, n_nodes), "left")
        deg = np.bincount(dst[lo:hi] - c * npc, minlength=pad)
        counts = np.zeros(nwin, np.int64)
        heap = [(0, w) for w in range(nwin)]
        heapq.heapify(heap)
        for nloc in np.argsort(-deg, kind="stable"):
            while True:
                s, w = heapq.heappop(heap)
                if counts[w] < W:
                    break
            slot_map[c, nloc] = w * W + counts[w]
            counts[w] += 1
            heapq.heappush(heap, (s + int(deg[nloc]), w))

    tcore = src // npc
    g_row = tcore * pad + slot_map[tcore, src - tcore * npc]

    # per (core, window) edge lists; find the global max tiles-per-window
    cores = []
    tpw = 1
    for c in range(n_cores):
        lo = np.searchsorted(dst, c * npc, "left")
        hi = np.searchsorted(dst, min((c + 1) * npc, n_nodes), "left")
        dloc = slot_map[c, dst[lo:hi] - c * npc]
        rows = g_row[lo:hi]
        o2 = np.argsort(dloc, kind="stable")
        dloc = dloc[o2]
        rows = rows[o2]
        win = dloc // W
        # edges are slot-sorted so windows are contiguous runs
        bounds = np.searchsorted(win, np.arange(nwin + 1))
        counts = bounds[1:] - bounds[:-1]
        tpw = max(tpw, int(math.ceil(counts.max() / 128)) if len(counts) else 1)
        cores.append((dloc, rows, bounds))

    cht = WPC * tpw               # tiles per chunk
    data = []
    for c in range(n_cores):
        dloc, rows, bounds = cores[c]
        idx32 = np.zeros((128, nch, cht), np.int32)
        ind = np.zeros((nch, 128, cht, W), np.float16)
        indt = np.zeros((nch, 128, cht // GRP, 128), np.float16)
        for iw in range(pad // W):
            lo, hi = bounds[iw], bounds[iw + 1]
            if hi == lo:
                continue
            ch, wloc = divmod(iw, WPC)
            for t in range(int(math.ceil((hi - lo) / 128))):
                s = lo + t * 128
                e = min(s + 128, hi)
                kk = e - s
                j = wloc * tpw + t
                idx32[:kk, ch, j] = rows[s:e]
                d_rel = dloc[s:e] - W * iw
                ar = np.arange(kk)
                ind[ch, ar, j, d_rel] = 1.0
                indt[ch, W * (j % GRP) + d_rel, j // GRP, ar] = 1.0
        data.append(dict(idx32=idx32, ind=ind, indt=indt))

    sel = np.zeros((WPC, cht), np.float16)
    for j in range(cht):
        sel[j // tpw, j] = 1.0
    return data, sel, nch, tpw, npc, pad, slot_map


# ---------------------------------------------------------------------------
# Device program
# ---------------------------------------------------------------------------

def split_excess_waits(nc, cap=1):
    """This walrus build accepts at most `cap` sync waits per instruction;
    split the extras onto preceding same-engine NOPs."""
    for fn in nc.m.functions:
        for blk in fn.blocks:
            lst = list(blk.instructions)
            changed = False
            i = 0
            while i < len(lst):
                inst = lst[i]
                si = inst.sync_info
                if si is not None and len(si.on_wait) > cap:
                    w = list(si.on_wait)
                    nop = mybir.InstNoOp(
                        name=nc.get_next_instruction_name(), engine=inst.engine,
                        sync_info=mybir.SyncInfo(on_wait=w[:cap], on_update=[]))
                    inst.sync_info = mybir.SyncInfo(
                        on_wait=w[cap:], on_update=list(si.on_update))
                    lst.insert(i, nop)
                    changed = True
                i += 1
            if changed:
                blk.instructions = lst


NSWQ = 1  # SWDGE queues: alternate gather instructions across queues
INTERLEAVE = False  # run flush work inside the edge-phase chunk loop
PSM_BUFS = 1
PS_BUFS = 4
SB3_BUFS = 3
G_BUFS = 3   # dedicated buffer depth for the gather destination tiles
DMA_SCRATCH = 16384  # SWDGE descriptor ring bytes/partition (default 16384)
EDGE_MODE = "full"   # "full" | "gather_only" | "no_gather" (timing ablations)
NOCOLL = False       # timing ablation: replace collectives with local no-ops
CHUNKED_AG = False    # AllGather table pieces overlapped with their producers


def build_nc(nch, tpw, pad, d_in, d_hid, d_out, n_cores=CORES):
    assert pad % WIN == 0 and pad % DST_PC == 0
    nblk = pad // 128
    wins = pad // WIN
    gt = pad * n_cores
    cht = WPC * tpw
    M1 = d_hid + 1
    M2 = d_out + 1
    R1 = d_hid + 8   # table1 row: h(64) | asrc | pad7 -> 144B
    R2 = d_out + 8   # table2 row: h2(32) | asrc2 | pad7 -> 80B
    AluOp = mybir.AluOpType
    Act = mybir.ActivationFunctionType
    rg = [list(range(n_cores))]

    nc = bass.Bass(num_swdge_queues=NSWQ,
                   dynamic_dma_scratch_size=DMA_SCRATCH)

    x = nc.dram_tensor("x", [pad, d_in], FP32, kind="ExternalInput")
    W1 = nc.dram_tensor("W1", [d_in, d_hid], FP32, kind="ExternalInput")
    a1 = nc.dram_tensor("a1", [d_hid, 2], FP32, kind="ExternalInput")
    b1 = nc.dram_tensor("b1", [1, d_hid], FP32, kind="ExternalInput")
    W2 = nc.dram_tensor("W2", [d_hid, d_out], FP32, kind="ExternalInput")
    a2 = nc.dram_tensor("a2", [d_out, 2], FP32, kind="ExternalInput")
    b2 = nc.dram_tensor("b2", [1, d_out], FP32, kind="ExternalInput")
    idx_s = nc.dram_tensor("idx_s", [128, nch, cht], I32, kind="ExternalInput")
    ind_s = nc.dram_tensor("ind_s", [nch, 128, cht, W], FP16, kind="ExternalInput")
    indt_s = nc.dram_tensor("indt_s", [nch, 128, cht // GRP, 128], FP16, kind="ExternalInput")
    sel_in = nc.dram_tensor("sel", [WPC, cht], FP16, kind="ExternalInput")
    out = nc.dram_tensor("out", [pad, d_out], FP32, kind="ExternalOutput")

    with TileContext(nc) as tc:
        with (
            tc.tile_pool(name="const", bufs=1) as constp,
            tc.tile_pool(name="sb", bufs=2) as sb,
            tc.tile_pool(name="sb3", bufs=SB3_BUFS) as sb3,
            tc.tile_pool(name="gp", bufs=G_BUFS) as gp,
            tc.tile_pool(name="ps", bufs=PS_BUFS, space="PSUM") as ps,
            tc.tile_pool(name="psm", bufs=PSM_BUFS, space="PSUM") as psm,
            tc.tile_pool(name="fps", bufs=2, space="PSUM") as fps,
            tc.tile_pool(name="dram", bufs=1, space="DRAM") as dr,
        ):
            ident = constp.tile([128, 128], FP32)
            make_identity(nc, ident[:])
            identh = constp.tile([128, 128], FP16)
            nc.vector.tensor_copy(identh[:], ident[:])
            ones_row = constp.tile([1, 128], FP16)
            nc.gpsimd.memset(ones_row[:], 1.0)
            qmask = constp.tile([128, 1, GRP], FP16)
            nc.gpsimd.memset(qmask[:], 0.0)
            for a in range(GRP):
                nc.gpsimd.memset(qmask[W * a:W * (a + 1), 0:1, a:a + 1], 1.0)
            selb = constp.tile([WPC, cht], FP16)
            nc.sync.dma_start(out=selb[:], in_=sel_in[:])

            tab1_sh = dr.tile([pad, R1], FP16)
            tab2_sh = dr.tile([pad, R2], FP16)
            nag1 = 4 if (nblk % 4 == 0 and CHUNKED_AG) else 1
            nag2 = 5 if (wins % 5 == 0 and CHUNKED_AG) else 1
            if CHUNKED_AG:
                tab1 = dr.tile([gt, R1], FP16)
                tab2 = dr.tile([gt, R2], FP16)
                tab1p = [dr.tile([gt // nag1, R1], FP16, addr_space="Shared",
                                 name=f"tab1p{p}") for p in range(nag1)]
                tab2p = [dr.tile([gt // nag2, R2], FP16, addr_space="Shared",
                                 name=f"tab2p{p}") for p in range(nag2)]
            else:
                tab1 = dr.tile([gt, R1], FP16, addr_space="Shared")
                tab2 = dr.tile([gt, R2], FP16, addr_space="Shared")
            adst1 = dr.tile([pad // W, W], FP16)
            adst2 = dr.tile([pad // W, W], FP16)

            acc1 = constp.tile([128, nblk, M1], FP32)
            acc2 = constp.tile([128, nblk, M2], FP32)

            # ---- weight prep ----
            w1sb = sb.tile([d_in, d_hid], FP16)
            nc.gpsimd.dma_start(out=w1sb[:], in_=W1[:])
            w1ext = constp.tile([d_in, d_hid + 2], FP16)
            nc.vector.tensor_copy(w1ext[:, :d_hid], w1sb[:])
            w1t_ps = ps.tile([128, 1024], FP16, tag="scr", name="w1t_ps")[:d_hid, :d_in]
            nc.tensor.transpose(w1t_ps[:], w1sb[:], identh[:d_in, :d_in])
            w1t = sb.tile([d_hid, d_in], FP16)
            nc.vector.tensor_copy(w1t[:], w1t_ps[:])
            a1sb = sb.tile([d_hid, 2], FP16)
            nc.gpsimd.dma_start(out=a1sb[:], in_=a1[:])
            wa_ps = ps.tile([128, 512], FP32, tag="scr", name="wa_ps")[:d_in, :2]
            nc.tensor.matmul(wa_ps[:], w1t[:], a1sb[:], start=True, stop=True)
            nc.vector.tensor_copy(w1ext[:, d_hid:d_hid + 2], wa_ps[:])

            w2sb = sb.tile([d_hid, d_out], FP16)
            nc.gpsimd.dma_start(out=w2sb[:], in_=W2[:])
            w2ext = constp.tile([d_hid, d_out + 2], FP16)
            nc.vector.tensor_copy(w2ext[:, :d_out], w2sb[:])
            w2t_ps = ps.tile([128, 1024], FP16, tag="scr", name="w2t_ps")[:d_out, :d_hid]
            nc.tensor.transpose(w2t_ps[:], w2sb[:], identh[:d_hid, :d_hid])
            w2t = sb.tile([d_out, d_hid], FP16)
            nc.vector.tensor_copy(w2t[:], w2t_ps[:])
            a2sb = sb.tile([d_out, 2], FP16)
            nc.gpsimd.dma_start(out=a2sb[:], in_=a2[:])
            wa2_ps = ps.tile([128, 512], FP32, tag="scr", name="wa2_ps")[:d_hid, :2]
            nc.tensor.matmul(wa2_ps[:], w2t[:], a2sb[:], start=True, stop=True)
            nc.vector.tensor_copy(w2ext[:, d_out:d_out + 2], wa2_ps[:])

            b1row = sb.tile([1, d_hid], FP16)
            nc.gpsimd.dma_start(out=b1row[:], in_=b1[:])
            b1_ps = ps.tile([128, 512], FP32, tag="scr", name="b1_ps")[:, :d_hid]
            nc.tensor.matmul(b1_ps[:], ones_row[:], b1row[:], start=True, stop=True)
            b1rep = constp.tile([128, d_hid], FP32)
            nc.vector.tensor_copy(b1rep[:], b1_ps[:])
            b2row = sb.tile([1, d_out], FP16)
            nc.gpsimd.dma_start(out=b2row[:], in_=b2[:])
            b2_ps = ps.tile([128, 512], FP32, tag="scr", name="b2_ps")[:, :d_out]
            nc.tensor.matmul(b2_ps[:], ones_row[:], b2row[:], start=True, stop=True)
            b2rep = constp.tile([128, d_out], FP32)
            nc.vector.tensor_copy(b2rep[:], b2_ps[:])

            def reduce_part_max(rm, nm):
                """[128, 2] fp32 -> [1, 2] max over partitions (PE transpose)."""
                rt_ps = ps.tile([128, 512], FP32, tag="scr", name=f"rt_ps{nm}")[:2, :128]
                nc.tensor.transpose(rt_ps[:], rm[:], ident[:])
                rt = sb.tile([2, 128], FP32, name=f"rt{nm}")
                nc.vector.tensor_copy(rt[:], rt_ps[:])
                rmx = sb.tile([2, 1], FP32, name=f"rmx{nm}")
                nc.vector.tensor_reduce(rmx[:], rt[:], mybir.AxisListType.X,
                                        op=AluOp.max)
                rmxh = sb.tile([2, 1], FP32, name=f"rmxh{nm}")
                nc.vector.tensor_copy(rmxh[:], rmx[:])
                bm_ps = ps.tile([128, 512], FP32, tag="scr", name=f"bm_ps{nm}")[:1, :2]
                nc.tensor.transpose(bm_ps[:], rmxh[:], ident[:2, :2])
                bout = sb.tile([1, 2], FP32, name=f"bout{nm}")
                nc.vector.tensor_copy(bout[:], bm_ps[:])
                return bout

            def make_negb_sb(bsb, nm):
                """exp-shift bias from an SBUF [1, 2] (asrc_max, adst_max).
                B only has to upper-bound this core's logits (the shift
                cancels in the per-dst softmax, which is core-local), so the
                core-local max suffices — no collective needed. fp16 exp
                leaves ~e^6 headroom over the cross-core max-statistic gap."""
                bsum = sb.tile([1, 1], FP32, name=f"bsum{nm}")
                nc.vector.tensor_add(bsum[:], bsb[:, 0:1], bsb[:, 1:2])
                bh = sb.tile([1, 1], FP16, name=f"bh{nm}")
                nc.vector.tensor_copy(bh[:], bsum[:])
                nb_ps = ps.tile([128, 512], FP32, tag="scr", name=f"nb_ps{nm}")[:, :1]
                nc.tensor.matmul(nb_ps[:], ones_row[:], bh[:], start=True, stop=True)
                negb = constp.tile([128, 1], FP32, name=f"negb{nm}")
                nc.vector.tensor_scalar(negb[:], nb_ps[:], -1.0, B_MARGIN,
                                        AluOp.mult, AluOp.add)
                return negb

            def ag_piece(tab_sh, tabp, tab, nag, p, R):
                """AllGather one finished table piece into its Shared buffer,
                then lay it into the flat rank-major gather table locally."""
                s = pad // nag
                nc.gpsimd.collective_compute(
                    "AllGather", AluOp.bypass, replica_groups=rg,
                    ins=[tab_sh[p * s:(p + 1) * s, :]], outs=[tabp[p][:]])
                nc.sync.dma_start(
                    out=tab[:].rearrange("(c q) r -> c q r", c=n_cores)
                        [:, p * s:(p + 1) * s, :],
                    in_=tabp[p][:].rearrange("(c q) r -> c q r", c=n_cores))

            # ---- phase X: transform nodes, build table1 ----
            runmax = sb.tile([128, 2], FP32)
            nc.vector.memset(runmax[:], -1e30)
            for blk in range(nblk):
                xb32 = sb3.tile([128, d_in], FP32, tag="xb32")
                nc.sync.dma_start(out=xb32[:], in_=x[blk * 128:(blk + 1) * 128, :])
                xb = sb3.tile([128, d_in], FP16, tag="xb")
                nc.vector.tensor_copy(xb[:], xb32[:])
                xt_ps = ps.tile([128, 1024], FP16, tag="scr", name="xt_ps")[:d_in, :128]
                nc.tensor.transpose(xt_ps[:], xb[:], identh[:])
                xt = sb3.tile([d_in, 128], FP16, tag="xt")
                nc.vector.tensor_copy(xt[:], xt_ps[:])
                he_ps = ps.tile([128, 512], FP32, tag="scr", name="he_ps")[:, :d_hid + 2]
                nc.tensor.matmul(he_ps[:], xt[:], w1ext[:], start=True, stop=True)
                row = sb3.tile([128, R1], FP16, tag="row")
                nc.vector.tensor_copy(row[:, :d_hid + 1], he_ps[:, :d_hid + 1])
                nc.vector.tensor_copy(
                    row[:, d_hid + 1:],
                    he_ps[:, d_hid + 1:d_hid + 2].to_broadcast([128, R1 - d_hid - 1]))
                nc.sync.dma_start(out=tab1_sh[blk * 128:(blk + 1) * 128, :], in_=row[:])
                ad = sb3.tile([128, 1], FP16, tag="ad")
                nc.vector.tensor_copy(ad[:], he_ps[:, d_hid + 1:d_hid + 2])
                nc.sync.dma_start(out=adst1[blk * (128 // W):(blk + 1) * (128 // W), :],
                                  in_=ad[:, 0])
                nc.vector.tensor_max(runmax[:], runmax[:], he_ps[:, d_hid:d_hid + 2])
                if not NOCOLL and CHUNKED_AG and (blk + 1) % (nblk // nag1) == 0:
                    ag_piece(tab1_sh, tab1p, tab1, nag1,
                             blk // (nblk // nag1), R1)

            bmax_sb = reduce_part_max(runmax, "1")
            if not NOCOLL and not CHUNKED_AG:
                nc.gpsimd.collective_compute(
                    "AllGather", AluOp.bypass, replica_groups=rg,
                    ins=[tab1_sh[:]], outs=[tab1[:]])

            negb1 = make_negb_sb(bmax_sb, "1")

            gq = [0]

            idxall = constp.tile([128, nch, cht], I32)
            nc.sync.dma_start(out=idxall[:], in_=idx_s[:])

            def edge_phase(tab, adst_t, acc, negb, d, M, R, after_chunk=None):
                for c in range(0 if EDGE_MODE == "none" else nch):
                    g = gp.tile([128, cht, R], FP16, tag="g")
                    if EDGE_MODE != "no_gather":
                        for j in range(cht):
                            gi = nc.gpsimd.indirect_dma_start(
                                out=g[:, j, :], out_offset=None, in_=tab[:],
                                in_offset=IndirectOffsetOnAxis(
                                    ap=idxall[:, c, j:j + 1], axis=0))
                            q = gq[0] % NSWQ
                            if q:
                                gi.ins.queue = f"qPoolDynamic{q}"
                            gq[0] += 1
                    else:
                        nc.vector.memset(g[:, 0, :], 0.125)
                    if EDGE_MODE == "gather_only":
                        dummy = sb3.tile([128, 1], FP16, tag="dummy")
                        nc.vector.tensor_copy(dummy[:], g[:, 0, 0:1])
                        acc_w = acc[:, c * BPC, 0:1]
                        nc.vector.tensor_copy(acc_w, dummy[:])
                        continue
                    indb = sb3.tile([128, cht, W], FP16, tag="ind")
                    nc.sync.dma_start(out=indb[:], in_=ind_s[c])
                    indtb = sb3.tile([128, cht // GRP, 128], FP16, tag="indt")
                    nc.sync.dma_start(out=indtb[:], in_=indt_s[c])

                    # per-window a_dst scores -> per-edge expansion
                    adsl = sb3.tile([WPC, W], FP16, tag="adsl")
                    nc.sync.dma_start(out=adsl[:], in_=adst_t[c * WPC:(c + 1) * WPC, :])
                    pb_t = sb3.tile([WPC, GRP, W], FP16, tag="pb_t")
                    nc.vector.tensor_copy(
                        pb_t[:], adsl[:, None, :].to_broadcast([WPC, GRP, W]))
                    ar_ps = ps.tile([128, 512], FP32, tag="scr", name="ar_ps")[:, :cht]
                    nc.tensor.matmul(ar_ps[:], pb_t[:].rearrange("w a d -> w (a d)"),
                                     selb[:], start=True, stop=True)
                    arhs = sb3.tile([128, cht // GRP, GRP], FP16, tag="arhs")
                    nc.vector.tensor_tensor(
                        arhs[:], ar_ps[:].rearrange("p (g a) -> p g a", a=GRP),
                        qmask[:].to_broadcast([128, cht // GRP, GRP]), AluOp.mult)
                    ex_ps = ps.tile([128, 512], FP32, tag="scr", name="ex_ps")[:, :cht]
                    for gi in range(cht // GRP):
                        nc.tensor.matmul(
                            ex_ps[:, GRP * gi:GRP * (gi + 1)],
                            indtb[:, gi, :], arhs[:, gi, :],
                            start=True, stop=True)

                    # logits -> weights
                    tbuf = sb3.tile([128, cht], FP32, tag="tbuf")
                    nc.vector.tensor_add(tbuf[:], g[:, :, d], ex_ps[:])
                    t02 = sb3.tile([128, cht], FP32, tag="t02")
                    nc.vector.tensor_scalar_mul(t02[:], tbuf[:], 0.2)
                    ubuf = sb3.tile([128, cht], FP32, tag="ubuf")
                    nc.vector.tensor_max(ubuf[:], tbuf[:], t02[:])
                    wbuf = sb3.tile([128, cht, 1], FP16, tag="wbuf")
                    nc.scalar.activation(wbuf[:, :, 0], ubuf[:], Act.Exp,
                                         bias=negb[:], scale=1.0)
                    wh = sb3.tile([128, cht, M], FP16, tag="wh")
                    nc.vector.tensor_tensor(
                        wh[:, :, :d], g[:, :, :d],
                        wbuf[:].to_broadcast([128, cht, d]), AluOp.mult)
                    nc.vector.tensor_copy(wh[:, :, d:d + 1], wbuf[:])

                    # message + segment-sum accumulation in PSUM: one psum
                    # tile per block, its two 64-row windows at bases 0/64.
                    pma = [psm.tile([128, 512], FP32, tag=f"pma{b}",
                                    name=f"pma{b}") for b in range(BPC)]
                    for j in range(cht):
                        wg = j // tpw
                        t = j % tpw
                        b, wl = divmod(wg, WPB)
                        nc.tensor.matmul(
                            pma[b][W * wl:W * (wl + 1), :M],
                            indb[:, j, :], wh[:, j, :],
                            start=(t == 0), stop=(t == tpw - 1))
                    for b in range(BPC):
                        nc.vector.tensor_copy(acc[:, c * BPC + b, :],
                                              pma[b][:, :M])
                    if after_chunk is not None:
                        after_chunk(c)

            # ---- flush layer 1 (interleaved into edge phase 1) ----
            runmax2 = sb.tile([128, 2], FP32)
            nc.vector.memset(runmax2[:], -1e30)

            def flush1_win(wk):
                h1t = sb3.tile([d_hid, WIN], FP16, tag="h1t")
                for t in range(4):
                    blk = wk * 4 + t
                    sc_col = sb3.tile([128, 1], FP32, tag="sc_col")
                    nc.vector.tensor_scalar_add(sc_col[:], acc1[:, blk, d_hid:d_hid + 1],
                                                1e-16)
                    rec = sb3.tile([128, 1], FP32, tag="rec")
                    nc.vector.reciprocal(rec[:], sc_col[:])
                    z = sb3.tile([128, d_hid], FP32, tag="z")
                    nc.vector.scalar_tensor_tensor(
                        z[:], acc1[:, blk, :d_hid], rec[:], b1rep[:],
                        AluOp.mult, AluOp.add)
                    h1r = sb3.tile([128, d_hid], FP16, tag="h1r")
                    nc.scalar.activation(h1r[:], z[:], Act.Relu)
                    h1t_ps = fps.tile([128, 1024], FP16, tag="fscr", name="h1t_ps")[:d_hid, :128]
                    nc.tensor.transpose(h1t_ps[:], h1r[:], identh[:])
                    nc.vector.tensor_copy(h1t[:, t * 128:(t + 1) * 128], h1t_ps[:])
                h2e_ps = fps.tile([128, 512], FP32, tag="fscr", name="h2e_ps")[:d_out + 2, :]
                nc.tensor.matmul(h2e_ps[:], w2ext[:], h1t[:], start=True, stop=True)
                v = sb3.tile([d_out + 2, WIN], FP32, tag="v")
                nc.vector.tensor_copy(v[:], h2e_ps[:])
                for t in range(4):
                    vt_ps = fps.tile([128, 512], FP32, tag="fscr", name="vt_ps")[:, :d_out + 2]
                    nc.tensor.transpose(vt_ps[:], v[:, t * 128:(t + 1) * 128],
                                        ident[:d_out + 2, :d_out + 2])
                    row2 = sb3.tile([128, R2], FP16, tag="row2")
                    nc.vector.tensor_copy(row2[:, :d_out + 1], vt_ps[:, :d_out + 1])
                    nc.vector.tensor_copy(
                        row2[:, d_out + 1:],
                        vt_ps[:, d_out + 1:d_out + 2].to_broadcast([128, R2 - d_out - 1]))
                    r0 = wk * WIN + t * 128
                    nc.sync.dma_start(out=tab2_sh[r0:r0 + 128, :], in_=row2[:])
                    ad2 = sb3.tile([128, 1], FP16, tag="ad2")
                    nc.vector.tensor_copy(ad2[:], vt_ps[:, d_out + 1:d_out + 2])
                    nc.sync.dma_start(out=adst2[r0 // W:r0 // W + 128 // W, :],
                                      in_=ad2[:, 0])
                    nc.vector.tensor_max(runmax2[:], runmax2[:],
                                         vt_ps[:, d_out:d_out + 2])

            # flush window wk needs blocks 4wk..4wk+4 = chunks 2wk, 2wk+1
            if INTERLEAVE:
                edge_phase(tab1, adst1, acc1, negb1, d_hid, M1, R1,
                           after_chunk=lambda c: flush1_win((c - 1) // 2)
                           if c % 2 == 1 else None)
            else:
                edge_phase(tab1, adst1, acc1, negb1, d_hid, M1, R1)
                for wk in range(wins):
                    flush1_win(wk)
                    if not NOCOLL and CHUNKED_AG and (wk + 1) % (wins // nag2) == 0:
                        ag_piece(tab2_sh, tab2p, tab2, nag2,
                                 wk // (wins // nag2), R2)

            bmax_sb2 = reduce_part_max(runmax2, "2")
            if not NOCOLL and not CHUNKED_AG:
                nc.gpsimd.collective_compute(
                    "AllGather", AluOp.bypass, replica_groups=rg,
                    ins=[tab2_sh[:]], outs=[tab2[:]])
            negb2 = make_negb_sb(bmax_sb2, "2")

            # ---- flush layer 2: log_softmax (interleaved into edge phase 2) ----
            def flush2_blk(blk):
                sl = slice(blk * 128, (blk + 1) * 128)
                sc2c = sb3.tile([128, 1], FP32, tag="sc2c")
                nc.vector.tensor_scalar_add(sc2c[:], acc2[:, blk, d_out:d_out + 1],
                                            1e-16)
                rec2 = sb3.tile([128, 1], FP32, tag="rec2")
                nc.vector.reciprocal(rec2[:], sc2c[:])
                z2 = sb3.tile([128, d_out], FP32, tag="z2")
                nc.vector.scalar_tensor_tensor(
                    z2[:], acc2[:, blk, :d_out], rec2[:], b2rep[:],
                    AluOp.mult, AluOp.add)
                mx = sb3.tile([128, 1], FP32, tag="mx")
                nc.vector.tensor_reduce(mx[:], z2[:], mybir.AxisListType.X,
                                        op=AluOp.max)
                nmx = sb3.tile([128, 1], FP32, tag="nmx")
                nc.vector.tensor_scalar_mul(nmx[:], mx[:], -1.0)
                es = sb3.tile([128, d_out], FP32, tag="es")
                sume = sb3.tile([128, 1], FP32, tag="sume")
                nc.scalar.activation(es[:], z2[:], Act.Exp, bias=nmx[:],
                                     scale=1.0, accum_out=sume[:])
                lns = sb3.tile([128, 1], FP32, tag="lns")
                nc.scalar.activation(lns[:], sume[:], Act.Ln)
                tot = sb3.tile([128, 1], FP32, tag="tot")
                nc.vector.tensor_add(tot[:], mx[:], lns[:])
                fin = sb3.tile([128, d_out], FP32, tag="fin")
                nc.vector.scalar_tensor_tensor(
                    fin[:], z2[:], tot[:], tot[:].to_broadcast([128, d_out]),
                    AluOp.subtract, AluOp.bypass)
                nc.sync.dma_start(out=out[sl, :], in_=fin[:])

            def after2(c):
                flush2_blk(c * BPC)
                flush2_blk(c * BPC + 1)

            if INTERLEAVE:
                edge_phase(tab2, adst2, acc2, negb2, d_out, M2, R2,
                           after_chunk=after2)
            else:
                edge_phase(tab2, adst2, acc2, negb2, d_out, M2, R2)
                for blk in range(nblk):
                    flush2_blk(blk)

    return nc


# ---------------------------------------------------------------------------
# Entry point
# ---------------------------------------------------------------------------

_CACHE = {}


class SpmdRunner:
    """Build the jitted 8-core executable once; reuse across calls."""

    def __init__(self, nc, n_cores):
        import jax
        from jax.sharding import Mesh, PartitionSpec
        from jax.experimental.shard_map import shard_map
        from concourse.bass2jax import (_bass_exec_p, install_neuronx_cc_hook,
                                        partition_id_tensor)
        install_neuronx_cc_hook()
        self.nc = nc
        self.n_cores = n_cores
        partition_name = nc.partition_id_tensor.name if nc.partition_id_tensor else None
        in_names, out_names, out_avals, zero_outs = [], [], [], []
        for alloc in nc.m.functions[0].allocations:
            if not isinstance(alloc, mybir.MemoryLocationSet):
                continue
            name = alloc.memorylocations[0].name
            if alloc.kind == "ExternalInput":
                if name != partition_name and name != (nc.dbg_addr.name if nc.dbg_addr else None):
                    in_names.append(name)
            elif alloc.kind == "ExternalOutput":
                out_names.append(name)
                shape = tuple(alloc.tensor_shape)
                dtype = mybir.dt.np(alloc.dtype)
                out_avals.append(jax.core.ShapedArray(shape, dtype))
                zero_outs.append(np.zeros(shape, dtype))
        self.in_names, self.out_names = in_names, out_names
        self.out_avals, self.zero_outs = out_avals, zero_outs
        n_params, n_outs = len(in_names), len(out_avals)
        all_in = in_names + out_names + ([partition_name] if partition_name else [])
        if nc.dbg_addr is not None:
            all_in.append(nc.dbg_addr.name)
        self.n_params = n_params

        def _body(*args):
            operands = list(args)
            if nc.dbg_addr is not None:
                operands.append(jax.numpy.zeros((1, 2), jax.numpy.uint32))
            if partition_name is not None:
                operands.append(partition_id_tensor())
            return tuple(_bass_exec_p.bind(
                *operands, out_avals=tuple(out_avals), in_names=tuple(all_in),
                out_names=tuple(out_names), lowering_input_output_aliases=(),
                sim_require_finite=True, sim_require_nnan=True, nc=nc))

        devices = jax.devices()[:n_cores]
        mesh = Mesh(np.asarray(devices), ("core",))
        self._mesh = mesh
        in_specs = (PartitionSpec("core"),) * (n_params + n_outs)
        out_specs = (PartitionSpec("core"),) * len(out_names)
        self._jax = jax
        # No donation: the zero "output seed" operands are unused by the NEFF
        # (outputs are separate buffers), so the same device-resident zeros
        # can be passed to every call, enabling pipelined timing runs.
        self._sharded = jax.jit(
            shard_map(_body, mesh=mesh, in_specs=in_specs, out_specs=out_specs,
                      check_rep=False),
            keep_unused=True)
        self._dz = None

    def prep_inputs(self, in_maps, device_resident=True):
        per_core = [[np.asarray(m[n]) for n in self.in_names] for m in in_maps]
        concat = [np.concatenate([per_core[c][i] for c in range(self.n_cores)], 0)
                  for i in range(self.n_params)]
        if not device_resident:
            return concat
        jax = self._jax
        from jax.sharding import NamedSharding, PartitionSpec
        sh = NamedSharding(self._mesh, PartitionSpec("core"))
        return [jax.device_put(a, sh) for a in concat]

    def _zeros(self):
        if self._dz is None:
            from jax.sharding import NamedSharding, PartitionSpec
            sh = NamedSharding(self._mesh, PartitionSpec("core"))
            self._dz = [self._jax.device_put(
                np.zeros((self.n_cores * z.shape[0], *z.shape[1:]), z.dtype), sh)
                for z in self.zero_outs]
        return self._dz

    def run(self, concat_in):
        out = self._sharded(*concat_in, *self._zeros())
        self._jax.block_until_ready(out)
        return out

    def run_pipelined(self, concat_in, n):
        """Issue n dispatches without intermediate sync; return wall seconds."""
        import time
        dz = self._zeros()
        t0 = time.time()
        outs = [self._sharded(*concat_in, *dz) for _ in range(n)]
        self._jax.block_until_ready(outs)
        return time.time() - t0

    def split_outputs(self, out_arrs):
        return [
            {n: np.asarray(out_arrs[i]).reshape(self.n_cores, *self.out_avals[i].shape)[c]
             for i, n in enumerate(self.out_names)}
            for c in range(self.n_cores)
        ]


def make_in_maps(x, W1, a_src1, a_dst1, b1, W2, a_src2, a_dst2, b2,
                 data, sel, npc, pad, n, slot_map):
    d_in = x.shape[1]
    xpad = np.zeros((CORES * pad, d_in), np.float32)
    for c in range(CORES):
        lo, hi = c * npc, min((c + 1) * npc, n)
        xpad[c * pad + slot_map[c, :hi - lo]] = x[lo:hi]
    in_maps = []
    for c in range(CORES):
        d = data[c]
        in_maps.append({
            "x": xpad[c * pad:(c + 1) * pad],
            "W1": np.asarray(W1, np.float32),
            "a1": np.stack([np.asarray(a_src1), np.asarray(a_dst1)], 1).astype(np.float32),
            "b1": np.asarray(b1, np.float32)[None, :],
            "W2": np.asarray(W2, np.float32),
            "a2": np.stack([np.asarray(a_src2), np.asarray(a_dst2)], 1).astype(np.float32),
            "b2": np.asarray(b2, np.float32)[None, :],
            "idx_s": d["idx32"], "ind_s": d["ind"], "indt_s": d["indt"],
            "sel": sel,
        })
    return in_maps


def kernel(x, edges_index, W1, a_src1, a_dst1, b1, W2, a_src2, a_dst2, b2):
    x = np.asarray(x, np.float32)
    edges_index = np.asarray(edges_index)
    n, d_in = x.shape
    d_hid = np.asarray(W1).shape[1]
    d_out = np.asarray(W2).shape[1]

    data, sel, nch, tpw, npc, pad, slot_map = preprocess(edges_index, n)

    key = (nch, tpw, pad, d_in, d_hid, d_out)
    if key not in _CACHE:
        nc = build_nc(*key)
        split_excess_waits(nc)
        _CACHE[key] = SpmdRunner(nc, CORES)
    r = _CACHE[key]

    in_maps = make_in_maps(x, W1, a_src1, a_dst1, b1, W2, a_src2, a_dst2, b2,
                           data, sel, npc, pad, n, slot_map)
    ci = r.prep_inputs(in_maps)
    outs = r.split_outputs(r.run(ci))
    res = np.concatenate(
        [outs[c]["out"][slot_map[c, :min((c + 1) * npc, n) - c * npc]]
         for c in range(CORES)], 0)
    return res.astype(np.float32)

